# revision 2
# baseline (speedup 1.0000x reference)
"""Trainium2 Bass kernel for nn_RahmanDynamicNet.

conv(1->20,(34,5)) -> BN(eval) -> sigmoid -> LIF(EMA) -> linear(20->1)
-> sigmoid -> LIF(EMA) -> [B,T] float32. T sharded across 8 cores (SPMD,
no collectives); host re-assembles.

Measured-rate-driven design (see microbench):
  - ACT = 0.6ns/col + ~190ns/instr -> sigmoids merged into 7 instructions
    via 4-PSUM-bank sweeps (2D APs over [4,480] f32).
  - DVE tensor_tensor bf16 = 0.16ns/col but tensor_reduce = 0.76ns/col ->
    the h-contraction is a 5-stage pairwise add tree of tensor_tensor ops.
  - DMA issue cost ~0.6us/instr dominates transfers -> the host pre-arranges
    x patches into the exact SBUF tile layout so each conv segment is ONE
    fully-contiguous 860KB DMA (4 total), weights/consts 4 more, out 1.
  - Conv as DoubleRow fp8e4m3 matmuls (both operands fp8): 420 patch rows
    in 2 chunks x [105 part, 2 ktile]; 240 streamed cols per 8-t block.
  - TL=512, WARM=12 (vs 576/44): exact-reset on core 0 via per-core scan
    boundary values DMA'd over on-device memset decay arrays.
  - p/v bf16; final x sw2 scale applied on host.
"""
import numpy as np
from contextlib import ExitStack
import sys

sys.path.insert(0, "/opt/trn_rl_repo")

import concourse.bass as bass
import concourse.bacc as bacc
import concourse.tile as tile
from concourse import mybir
from concourse.bass_utils import run_bass_kernel_spmd
import ml_dtypes

BF16 = ml_dtypes.bfloat16
E4M3 = ml_dtypes.float8_e4m3

B, F, T, H, K = 128, 34, 4000, 20, 5
NCORES = 8
S = 8              # outputs per block
JW = S + 4         # taps per block window (12)
FA = F + 1         # channels + ones row (35)
ROWS = JW * FA     # 420
NCH = 2            # DoubleRow chunks
TAPC = JW // NCH   # 6 taps per chunk
CHP = 3 * FA       # 105 partitions per chunk
NB = 64            # blocks per core
NSB = 4            # DMA segments
SBB = NB // NSB    # 16 blocks per segment
TL = NB * S        # 512
WARM = 12
TO = T // NCORES   # 500
NCOLS = S * H      # 160
PADL = 48
BN_EPS = 1e-5
GRP = 12           # blocks per sigmoid sweep (4 PSUM banks x 3)
BPB = 3            # blocks per PSUM bank

_DT = mybir.dt
DR = mybir.MatmulPerfMode.DoubleRow


def _sigmoid(v):
    return 1.0 / (1.0 + np.exp(-v))


def build_nc(sw1, sw2, linb, reps=1):
    nc = bacc.Bacc()
    xt = nc.declare_dram_parameter("xt", [NSB, CHP, NCH, 2, SBB, B],
                                   _DT.float8e4, isOutput=False)
    wcp = nc.declare_dram_parameter("wc", [CHP, NCH, 2, NCOLS], _DT.float8e4,
                                    isOutput=False)
    wlp = nc.declare_dram_parameter("wl", [B, H], _DT.bfloat16, isOutput=False)
    bvp = nc.declare_dram_parameter("bv", [B, 2], _DT.float32, isOutput=False)
    outp = nc.declare_dram_parameter("out", [B, TO], _DT.bfloat16,
                                    isOutput=True)

    with ExitStack() as ctx:
        tc = ctx.enter_context(tile.TileContext(nc))
        singles = ctx.enter_context(tc.tile_pool(name="singles", bufs=1))
        xp = ctx.enter_context(tc.tile_pool(name="xp", bufs=3))
        pp = ctx.enter_context(tc.tile_pool(name="pp", bufs=2, space="PSUM"))
        up = ctx.enter_context(tc.tile_pool(name="up", bufs=2))
        tp = ctx.enter_context(tc.tile_pool(name="tp", bufs=2))

        wc_sb = singles.tile([CHP, NCH, 2, NCOLS], _DT.float8e4)
        nc.scalar.dma_start(out=wc_sb, in_=wcp[:, :, :, :])
        wl_sb = singles.tile([B, H], _DT.bfloat16)
        nc.scalar.dma_start(out=wl_sb, in_=wlp[:, :])

        d0a_sb = singles.tile([B, TL], _DT.float32)
        d0b_sb = singles.tile([B, TL], _DT.float32)
        zb_sb = singles.tile([B, 1], _DT.float32)
        p_sb = singles.tile([B, TL], _DT.bfloat16)
        q_sb = singles.tile([B, TL], _DT.float32)
        z_sb = singles.tile([B, TL], _DT.float32)
        v_sb = singles.tile([B, TL], _DT.bfloat16)

        nc.vector.memset(d0a_sb[:, :], float(1.0 - sw1))
        nc.vector.memset(d0b_sb[:, :], float(1.0 - sw2))
        nc.vector.memset(zb_sb[:, :], float(linb))
        # per-core scan-boundary decay at col WARM (0 on core 0: exact reset)
        nc.scalar.dma_start(out=d0a_sb[:, WARM:WARM + 1], in_=bvp[:, 0:1])
        nc.scalar.dma_start(out=d0b_sb[:, WARM:WARM + 1], in_=bvp[:, 1:2])

        def wl_bcast(nrep):
            a = wl_sb[:, :]
            return bass.AP(tensor=a.tensor, offset=a.offset,
                           ap=[list(a.ap[0]), [0, nrep], [1, H]])

        for _rep in range(reps):
            xbs = []
            for seg in range(NSB):
                xb = xp.tile([CHP, NCH, 2, SBB, B], _DT.float8e4,
                             name=f"xb{seg}")
                nc.sync.dma_start(out=xb, in_=xt[seg, :, :, :, :, :])
                xbs.append(xb)

            # sweep groups: 5 x 12 blocks + tail 4 blocks
            sweeps = [(g * GRP, GRP) for g in range(NB // GRP)]
            if NB % GRP:
                sweeps.append((NB - NB % GRP, NB % GRP))
            for b0, nblk in sweeps:
                ps = pp.tile([B, 4, 512], _DT.float32)
                for e in range(nblk):
                    blk = b0 + e
                    xb = xbs[blk // SBB]
                    ibl = blk % SBB
                    bank, slot = e // BPB, e % BPB
                    co = slot * NCOLS
                    # mA: chunk1 full width, start=True clears the bank
                    # region (cols 0:40 are zero weights)
                    nc.tensor.matmul(
                        ps[:, bank, co:co + 160], xb[:, 1, :, ibl, :],
                        wc_sb[:, 1, :, 0:160],
                        start=True, stop=False, perf_mode=DR,
                        skip_group_check=True)
                    # mB: chunk0 accumulates outputs 0-5 (cols 0:120)
                    nc.tensor.matmul(
                        ps[:, bank, co:co + 120], xb[:, 0, :, ibl, :],
                        wc_sb[:, 0, :, 0:120],
                        start=False, stop=True, perf_mode=DR,
                        skip_group_check=True)
                # sigmoid sweep(s) + h-contraction tree
                if nblk == GRP:
                    chunks = [(0, 4)]     # one ACT instr over 4 banks
                else:
                    nfull, rem = nblk // BPB, nblk % BPB
                    chunks = [(0, nfull)] if nfull else []
                    if rem:
                        chunks.append((nfull, -rem))  # partial bank
                for bk0, nbk in chunks:
                    if nbk > 0:
                        nt = nbk * BPB * S
                        src = bass.AP(
                            tensor=ps.tensor,
                            offset=ps[:, bk0, 0:1].offset,
                            ap=[list(ps[:, 0, 0:1].ap[0]), [512, nbk],
                                [1, BPB * NCOLS]])
                    else:
                        nt = (-nbk) * S
                        src = ps[:, bk0, 0:(-nbk) * NCOLS]
                    t0 = S * b0 + bk0 * BPB * S
                    nh = nt * H
                    u_t = up.tile([B, nh], _DT.bfloat16, name="u_t")
                    nc.scalar.activation(
                        out=u_t[:, :], in_=src,
                        func=mybir.ActivationFunctionType.Sigmoid)
                    um = tp.tile([B, nt, H], _DT.bfloat16, name="um")
                    nc.vector.tensor_mul(um.rearrange("p t h -> p (t h)"),
                                         u_t[:, :], wl_bcast(nt))
                    with nc.allow_low_precision(reason="bf16 tree sums; "
                                                "validated vs gate"):
                        t1 = tp.tile([B, nt, 10], _DT.bfloat16, name="t1")
                        nc.vector.tensor_add(t1, um[:, :, 0:10],
                                             um[:, :, 10:20])
                        t2 = tp.tile([B, nt, 5], _DT.bfloat16, name="t2")
                        nc.vector.tensor_add(t2, t1[:, :, 0:5], t1[:, :, 5:10])
                        t3 = tp.tile([B, nt, 2], _DT.bfloat16, name="t3")
                        nc.vector.tensor_add(t3, t2[:, :, 0:2], t2[:, :, 2:4])
                        t4 = tp.tile([B, nt], _DT.bfloat16, name="t4")
                        nc.vector.tensor_add(t4, t3[:, :, 0], t3[:, :, 1])
                        nc.vector.tensor_add(p_sb[:, t0:t0 + nt], t4[:, :],
                                             t2[:, :, 4])

            # tail: q-scan -> z sigmoid -> v-scan (all on DVE/ACT), segmented
            SEGC = TL // NSB
            for seg in range(NSB):
                s0, s1 = SEGC * seg, SEGC * (seg + 1)
                nc.vector.tensor_tensor_scan(
                    out=q_sb[:, s0:s1], data0=d0a_sb[:, s0:s1],
                    data1=p_sb[:, s0:s1],
                    initial=(0.0 if seg == 0 else q_sb[:, s0 - 1:s0]),
                    op0=mybir.AluOpType.mult, op1=mybir.AluOpType.add)
                nc.scalar.activation(
                    out=z_sb[:, s0:s1], in_=q_sb[:, s0:s1],
                    func=mybir.ActivationFunctionType.Sigmoid,
                    bias=zb_sb[:, 0:1])
                with nc.allow_low_precision(reason="bf16 v; host sw2 scale"):
                    nc.vector.tensor_tensor_scan(
                        out=v_sb[:, s0:s1], data0=d0b_sb[:, s0:s1],
                        data1=z_sb[:, s0:s1],
                        initial=(0.0 if seg == 0 else v_sb[:, s0 - 1:s0]),
                        op0=mybir.AluOpType.mult, op1=mybir.AluOpType.add)
            nc.sync.dma_start(out=outp[:, :], in_=v_sb[:, WARM:WARM + TO])
    nc.compile()
    return nc


def prep(x, conv_w, conv_b, bn_gamma, bn_beta, bn_mean, bn_var,
         lin_w, lin_b, w1, w2):
    x = np.asarray(x, np.float32)
    inv = (np.asarray(bn_gamma, np.float32)
           / np.sqrt(np.asarray(bn_var, np.float32) + BN_EPS))
    shift = (np.asarray(conv_b, np.float32)
             - np.asarray(bn_mean, np.float32)) * inv \
        + np.asarray(bn_beta, np.float32)
    sw1 = float(_sigmoid(np.float32(np.asarray(w1))))
    sw2 = float(_sigmoid(np.float32(np.asarray(w2))))
    linb = float(np.asarray(lin_b, np.float32).reshape(-1)[0])
    lw = np.asarray(lin_w, np.float32).reshape(-1)

    GT = PADL + T + 64
    x_aug = np.zeros((GT, FA, B), np.float32)
    x_aug[PADL:PADL + T, :F, :] = x[:, 0].transpose(2, 1, 0)
    x_aug[PADL:PADL + T, F, :] = 1.0
    x_q = x_aug.astype(E4M3)

    cw = np.asarray(conv_w, np.float32)[:, 0]  # [H,F,K]
    Wf = np.zeros((ROWS, NCOLS), np.float32)
    for i in range(S):
        for k in range(K):
            j = i + k
            Wf[j * FA:j * FA + F, i * H:(i + 1) * H] = \
                (cw[:, :, k] * inv[:, None]).T
        Wf[(i + 2) * FA + F, i * H:(i + 1) * H] = shift
    # row (j,ch) -> chunk c=j//6, ktile kt=(j%6)//3, partition p=(j%3)*35+ch
    wc = np.zeros((CHP, NCH, 2, NCOLS), np.float32)
    for j in range(JW):
        c, kt, jl = j // TAPC, (j % TAPC) // 3, j % 3
        wc[FA * jl:FA * (jl + 1), c, kt, :] = Wf[j * FA:(j + 1) * FA, :]
    wc = wc.astype(E4M3)

    wl = np.ascontiguousarray(
        np.broadcast_to((lw * sw1).astype(BF16), (B, H)))

    # xt: pre-arranged patch slabs, one contiguous DMA per segment:
    # xt[seg, p=(jl*35+ch), c, kt, blk, b] = x_q[g0 + 128*seg + j + 8*blk, ch, b]
    JLv = np.arange(3)[:, None, None, None, None]        # jl
    CHv = np.arange(FA)[None, :, None, None, None]       # ch
    Cv = np.arange(NCH)[None, None, :, None, None]       # c
    KTv = np.arange(2)[None, None, None, :, None]        # kt
    BLKv = np.arange(SBB)[None, None, None, None, :]     # blk
    in_maps = []
    for core in range(NCORES):
        g0 = TO * core + PADL - WARM - 2
        xtc = np.empty((NSB, CHP, NCH, 2, SBB, B), E4M3)
        for seg in range(NSB):
            tidx = (g0 + 128 * seg + TAPC * Cv + 3 * KTv + JLv + S * BLKv)
            # gathered: [3, 35, 2, 2, 16, B]
            gath = x_q[tidx, CHv, :]
            xtc[seg] = gath.reshape(CHP, NCH, 2, SBB, B)
        bvc = np.zeros((B, 2), np.float32)
        bvc[:, 0] = 0.0 if core == 0 else (1.0 - sw1)
        bvc[:, 1] = 0.0 if core == 0 else (1.0 - sw2)
        in_maps.append({"xt": xtc, "wc": wc, "wl": wl, "bv": bvc})
    return in_maps, sw1, sw2, linb


_NC_CACHE = {}


def kernel(**inputs):
    in_maps, sw1, sw2, linb = prep(**inputs)
    key = (round(sw1, 9), round(sw2, 9), round(linb, 9))
    if key not in _NC_CACHE:
        _NC_CACHE[key] = build_nc(sw1, sw2, linb)
    nc = _NC_CACHE[key]
    res = run_bass_kernel_spmd(nc, in_maps, list(range(NCORES)))
    outs = [np.asarray(res.results[c]["out"], np.float32)
            for c in range(NCORES)]
    return np.concatenate(outs, axis=1) * np.float32(sw2)
